# revision 3
# baseline (speedup 1.0000x reference)
"""DistTokenMix kernel for Trainium2 (8 NeuronCores).

Math: out[b,i,d] = sum_j h[b,j,d] * alpha[spd[i,j], d]
     (B=8, N=4096, D=64, NUM_BUCKETS=8)

Equivalent to 8 masked NxN matmuls: out = sum_k (M_k @ h) * alpha[k],
with M_k[i,j] = [spd[i,j] == k].

Sharding: output rows i are sharded across the 8 cores (512 rows each).
Each core holds its spd row-block (as the transposed column-block, so the
contraction index j lands on SBUF partitions) plus all of h.

Per-core device kernel:
  - h is staged as [j, (b,d)] fp32 (dtype-labeled float32r -> tensor engine
    runs it single-pass at bf16 speed with ~2^-13 precision).
  - spd block arrives as int64 viewed as int32 pairs; an on-device strided
    copy extracts the low words as bf16 values (0..7 exact).
  - per bucket k: mask[j,i] = (spd_vals == k) as fp32r (0/1 exact);
    matmuls accumulate psum[bd_chunk][128, 512] over all j;
    fused drain acc[bd] = psum * alpha_vec[bd] + acc (scalar_tensor_tensor).
  - out[bd, i] per core; host reassembles to [B, N, D].
"""
import numpy as np

import concourse.bass as bass
import concourse.mybir as mybir
import concourse.tile as tile
from concourse import bacc
from concourse.bass_utils import run_bass_kernel_spmd

B, N, D = 8, 4096, 64
NB = 8              # buckets
NCORES = 8
IC = N // NCORES    # 512 output rows per core
BD = B * D          # 512
NJT = N // 128      # 32 j tiles
CHUNK_JT = 4        # j-tiles per spd DMA chunk
NCHUNK = NJT // CHUNK_JT

f32 = mybir.dt.float32
f32r = mybir.dt.float32r
bf16 = mybir.dt.bfloat16
i32 = mybir.dt.int32


def build_nc():
    nc = bacc.Bacc(trn_type="TRN2")
    # per-core column block of spd (transposed on host): int64 [N, IC] viewed
    # as int32 [N, 2*IC] (little-endian low word holds the value)
    spdT = nc.dram_tensor("spdT", [N, 2 * IC], i32, kind="ExternalInput")
    # h relayout [j, b*64+d], labeled fp32r (same bytes as fp32)
    hj = nc.dram_tensor("hj", [N, BD], f32r, kind="ExternalInput")
    # alpha_part[p, k] = alpha[k, p % 64]
    alphap = nc.dram_tensor("alphap", [128, NB], f32, kind="ExternalInput")
    out = nc.dram_tensor("out", [BD, IC], f32, kind="ExternalOutput")

    with tile.TileContext(nc) as tc:
        with (
            tc.tile_pool(name="persist", bufs=1) as persist,
            tc.tile_pool(name="stage", bufs=2) as stagep,
            tc.tile_pool(name="maskp", bufs=3) as maskp,
            tc.tile_pool(name="psum", bufs=2, space="PSUM") as psump,
        ):
            # ---- persistent tiles ----
            h_sb = persist.tile([128, NJT, BD], f32r)        # 64 KB/part
            vals = persist.tile([128, NJT, IC], bf16)        # 32 KB/part
            beta = persist.tile([128, NB], f32)
            accs = [persist.tile([128, IC], f32, name=f"acc{c}", tag=f"acc{c}") for c in range(4)]

            nc.sync.dma_start(beta[:], alphap[:])
            # h: [N, BD] -> [128, (jt, bd)]
            nc.sync.dma_start(h_sb[:], hj.rearrange("(t p) w -> p t w", p=128))

            # ---- spd load + compact to bf16 values ----
            spd_r = spdT.rearrange("(c t p) w -> c p t w", t=CHUNK_JT, p=128)
            for c in range(NCHUNK):
                stage = stagep.tile([128, CHUNK_JT, 2 * IC], i32)
                nc.sync.dma_start(stage[:], spd_r[c])
                # low int32 of each int64 -> bf16 (values 0..7, exact)
                nc.scalar.copy(
                    vals[:, c * CHUNK_JT:(c + 1) * CHUNK_JT, :],
                    stage[:, :, ::2],
                )

            # ---- bucket loop ----
            for k in range(NB):
                pss = [psump.tile([128, IC], f32, name=f"ps{ch}", tag=f"ps{ch}") for ch in range(4)]
                for c in range(NCHUNK):
                    mask = maskp.tile([128, CHUNK_JT, IC], f32r)
                    nc.vector.tensor_scalar(
                        mask[:], vals[:, c * CHUNK_JT:(c + 1) * CHUNK_JT, :],
                        float(k), None, mybir.AluOpType.is_equal,
                    )
                    for t in range(CHUNK_JT):
                        jt = c * CHUNK_JT + t
                        for ch in range(4):
                            nc.tensor.matmul(
                                pss[ch][:],
                                h_sb[:, jt, ch * 128:(ch + 1) * 128],
                                mask[:, t, :],
                                start=(jt == 0),
                                stop=(jt == NJT - 1),
                            )
                # drain: acc = psum * beta[:, k] (+ acc)
                for ch in range(4):
                    if k == 0:
                        nc.vector.tensor_scalar(
                            accs[ch][:], pss[ch][:], beta[:, k:k + 1], None,
                            mybir.AluOpType.mult,
                        )
                    else:
                        nc.vector.scalar_tensor_tensor(
                            accs[ch][:], pss[ch][:], beta[:, k:k + 1], accs[ch][:],
                            op0=mybir.AluOpType.mult, op1=mybir.AluOpType.add,
                        )

            for ch in range(4):
                nc.sync.dma_start(out[ch * 128:(ch + 1) * 128, :], accs[ch][:])

    nc.compile()
    return nc


_NC_CACHE = None


def _get_nc():
    global _NC_CACHE
    if _NC_CACHE is None:
        _NC_CACHE = build_nc()
    return _NC_CACHE


def _prep_inputs(h, spd, alpha):
    h = np.asarray(h, dtype=np.float32)
    alpha = np.asarray(alpha, dtype=np.float32)
    spd = np.asarray(spd)
    if spd.dtype != np.int64:
        spd = spd.astype(np.int64)
    # [j, b*64+d]
    h_jbd = np.ascontiguousarray(h.transpose(1, 0, 2).reshape(N, BD))
    alphap = np.ascontiguousarray(alpha[:, np.arange(128) % 64].T)  # [128, 8]
    in_maps = []
    for c in range(NCORES):
        blk = np.ascontiguousarray(spd[c * IC:(c + 1) * IC, :].T)  # [N, IC] int64
        spdT_i32 = blk.view(np.int32).reshape(N, 2 * IC)
        in_maps.append({"spdT": spdT_i32, "hj": h_jbd, "alphap": alphap})
    return in_maps


def _assemble(results):
    outs = []
    for c in range(NCORES):
        o = results[c]["out"]                       # [BD, IC] = [(b,d), i]
        outs.append(o.reshape(B, D, IC).transpose(0, 2, 1))  # [b, i_local, d]
    return np.ascontiguousarray(np.concatenate(outs, axis=1))  # [B, N, D]


def kernel(h, spd, alpha, _trace=False):
    nc = _get_nc()
    in_maps = _prep_inputs(h, spd, alpha)
    res = run_bass_kernel_spmd(nc, in_maps, list(range(NCORES)), trace=_trace)
    out = _assemble(res.results)
    if _trace:
        kernel.last_result = res
    return out
